# revision 21
# baseline (speedup 1.0000x reference)
"""BitLinear (ternary weight / int8-activation quantized matmul) Trainium2 kernel.

Reference semantics (for x:(B,S,D), weight:(O,D)):
    alpha = max(mean(|W|), 1e-8)                     # per-tensor scalar
    w_q   = clip(round(W/alpha), -1, 1)              # ternary
    beta  = max(max|x| / 127, 1e-8)                  # per token
    x_q   = clip(round(x/beta), -127, 127)           # int8 range
    y     = (x_q @ w_q.T) * alpha * beta

Sharding: data-parallel over the 16384 tokens across 8 NeuronCores
(2048 tokens/core); full weight replicated per core (no collectives —
a 4-byte AllReduce measures ~56us on this runtime, slower than just
reading W locally).

Quantized GEMM runs in fp16 which is EXACT here: x_q in [-127,127] and
w_q in {-1,0,1} are exactly representable and fp32 PSUM accumulation
of the integer partial sums is exact.

Rounding uses the fp16 magic-number trick: (v + 1536.0) computed in
fp32 inside an engine and converted to fp16 on output rounds v to the
nearest integer (RNE), since ulp(1536)=1 in fp16. x quantization is
thus a SINGLE scalar-engine activation pass producing xm = 1536 + x_q;
the PE-transpose evacuation pass removes the offset. (The W path
removes its offset during the clip passes — keeping offset values in
the GEMM operands trips the PE's HAM power throttle and slows every
matmul by ~30%.)

Schedule: W tiles are DMA'd first (whole-tile DMAs spread descriptors
across all 16 queues; ~2.9us per 1 MiB tile of added completion
latency) with |W| accumulation riding the stream on ScalarE, so alpha
is ready right after the last W tile lands. The post-alpha
quantization sprint is split across ScalarE/VectorE/GpSimd, and the
matmul loop accumulates both PSUM halves k-by-k so the PE starts on
wq[0] immediately and trickle-consumes the rest.
"""

import numpy as np

import bass_rust
import concourse.bass as bass
import concourse.mybir as mybir
import concourse.tile as tile
from concourse.bass_utils import run_bass_kernel_spmd
from concourse.masks import make_identity

N_CORES = 8
P = 128
M16 = 1536.0  # 1.5 * 2**10 : fp16 RNE round-to-integer magic constant
EPS = 1e-8

# Full-problem shapes (hardcoded per the grading contract)
FULL_B, FULL_S, FULL_D = 4, 4096, 2048
D_IN = 2048
D_OUT = 2048
TOK_PER_CORE = FULL_B * FULL_S // N_CORES  # 2048

WS_BUFS = 9    # W f32 tiles kept resident through the alpha pass; the
               # remaining 7 stream through a small pool for |W| accum
               # and are re-read after alpha (SBUF cannot hold all 16
               # f32 tiles plus the fp16 wq)


def _split_excess_waits(nc, max_waits=1):
    """This container's walrus accepts at most `max_waits` sync waits per
    instruction; move excess waits onto preceding same-engine nops."""
    n = 0
    for f in nc.m.functions:
        for bb in f.blocks:
            insts = list(bb.instructions)
            out = []
            changed = False
            for inst in insts:
                si = inst.sync_info
                if si is not None and len(si.on_wait) > max_waits:
                    waits = list(si.on_wait)
                    extra, keep = waits[:-max_waits], waits[-max_waits:]
                    for i in range(0, len(extra), max_waits):
                        chunk = extra[i : i + max_waits]
                        n += 1
                        nop = mybir.InstNoOp(name=f"waitsplit-{n}")
                        nop.engine = inst.engine
                        nop.sync_info = bass_rust.SyncInfo(
                            on_wait=chunk, on_update=[]
                        )
                        out.append(nop)
                    inst.sync_info = bass_rust.SyncInfo(
                        on_wait=keep, on_update=list(si.on_update)
                    )
                    changed = True
                out.append(inst)
            if changed:
                bb.instructions = out


def emit_bitlinear(tc, y_ap, x_ap, wt_ap, d_in, d_out, n_tok):
    """Emit the per-core kernel body.

    x_ap:  [n_tok, d_in]  f32 token rows for this core
    wt_ap: [d_in, d_out]  f32 transposed weight (wt[i,o] = W[o,i])
    y_ap:  [n_tok, d_out] f16 output (host upcasts to f32)
    """
    from contextlib import ExitStack

    nc = tc.nc
    f32 = mybir.dt.float32
    f16 = mybir.dt.float16
    bf16 = mybir.dt.bfloat16
    NK = d_in // P          # 16 contraction tiles
    NX = n_tok // P         # 16 token tiles
    inv_n = 1.0 / float(d_in * d_out)
    Id = mybir.ActivationFunctionType.Identity
    Ab = mybir.ActivationFunctionType.Abs

    with ExitStack() as ctx:
        const = ctx.enter_context(tc.tile_pool(name="const", bufs=1))
        ws = ctx.enter_context(tc.tile_pool(name="ws", bufs=WS_BUFS))
        ws2 = ctx.enter_context(tc.tile_pool(name="ws2", bufs=2))
        wqp = ctx.enter_context(tc.tile_pool(name="wqp", bufs=1))
        qtmp = ctx.enter_context(tc.tile_pool(name="qtmp", bufs=2))
        xf32 = ctx.enter_context(tc.tile_pool(name="xf32", bufs=2))
        xqm = ctx.enter_context(tc.tile_pool(name="xqm", bufs=2))
        xqtp = ctx.enter_context(tc.tile_pool(name="xqtp", bufs=2))
        ybf = ctx.enter_context(tc.tile_pool(name="ybf", bufs=1))
        small = ctx.enter_context(tc.tile_pool(name="small", bufs=16))
        pyp = ctx.enter_context(tc.tile_pool(name="pyp", bufs=3, space="PSUM"))
        ptp = ctx.enter_context(tc.tile_pool(name="ptp", bufs=1, space="PSUM"))
        pap = ctx.enter_context(tc.tile_pool(name="pap", bufs=1, space="PSUM"))

        ident = const.tile([P, P], f16)
        make_identity(nc, ident)
        ones_k = const.tile([P, 1], f32)
        nc.vector.memset(ones_k, 1.0)
        ones_m = const.tile([1, P], f32)
        nc.vector.memset(ones_m, 1.0)
        m16b = const.tile([P, 1], f32)
        nc.vector.memset(m16b, M16)
        partials = const.tile([P, NK], f32)
        wq = wqp.tile([P, NK, d_out], f16)

        # ---- head DMAs: x0 (for staging), then the full W stream ----
        x_in = {}
        xi0 = xf32.tile([P, d_in], f32, tag="xi", name="xi0")
        nc.sync.dma_start(out=xi0, in_=x_ap[0:P, :])
        x_in[0] = xi0
        wtile = {}
        for j in range(NK):
            if j < WS_BUFS:
                wj = ws.tile([P, d_out], f32, tag="ws", name=f"w{j}")
                wtile[j] = wj
            else:
                wj = ws2.tile([P, d_out], f32, tag="ws2", name=f"w{j}")
            nc.sync.dma_start(out=wj, in_=wt_ap[j * P : (j + 1) * P, :])
            # |W| row-sums ride the stream on ScalarE
            trash = qtmp.tile([P, d_out], bf16, tag="qm", name=f"trash{j}")
            nc.scalar.activation(
                out=trash, in_=wj, func=Ab, accum_out=partials[:, j : j + 1],
            )

        def x_dma(i):
            if i in x_in:
                return x_in.pop(i)
            xi = xf32.tile([P, d_in], f32, tag="xi", name=f"xi{i}")
            nc.sync.dma_start(out=xi, in_=x_ap[i * P : (i + 1) * P, :])
            return xi

        def x_stage(i):
            xi = x_dma(i)
            am = small.tile([P, 1], f32, tag="am", name=f"am{i}")
            nc.vector.tensor_reduce(
                out=am, in_=xi, axis=mybir.AxisListType.X,
                op=mybir.AluOpType.max, apply_absolute_value=True,
            )
            beta = small.tile([P, 1], f32, tag="beta", name=f"beta{i}")
            nc.vector.tensor_scalar(
                beta, am, 1.0 / 127.0, EPS,
                mybir.AluOpType.mult, mybir.AluOpType.max,
            )
            invb = small.tile([P, 1], f32, tag="invb", name=f"invb{i}")
            nc.vector.reciprocal(out=invb, in_=beta)
            # xm = fp16(x*invb + 1536) == 1536 + x_q  (exact RNE round)
            xm = xqm.tile([P, d_in], f16, tag="xm", name=f"xm{i}")
            nc.scalar.activation(out=xm, in_=xi, func=Id, scale=invb, bias=m16b)
            # transpose in 2 groups of 8; evacuation removes the offset
            xqt = xqtp.tile([P, NK, P], f16, tag="xqt", name=f"xqt{i}")
            for g in range(2):
                pt = ptp.tile([P, 8, P], f16, tag="pt", name=f"pt{i}_{g}")
                for jj in range(8):
                    k = g * 8 + jj
                    nc.tensor.transpose(
                        pt[:, jj, :], xm[:, k * P : (k + 1) * P], ident
                    )
                nc.vector.tensor_scalar(
                    xqt[:, g * 8 : (g + 1) * 8, :], pt, M16, None,
                    mybir.AluOpType.subtract,
                )
            return beta, xqt

        staged = {}
        staged[0] = x_stage(0)

        # ---- alpha = max(mean|W|, EPS); broadcast alpha & 1/alpha ----
        total = const.tile([P, 1], f32)
        nc.vector.tensor_reduce(
            out=total, in_=partials, axis=mybir.AxisListType.X,
            op=mybir.AluOpType.add,
        )
        pa_sum = pap.tile([1, 1], f32, tag="pa")
        nc.tensor.matmul(pa_sum, lhsT=total, rhs=ones_k, start=True, stop=True)
        scal = const.tile([1, 2], f32)
        nc.vector.tensor_scalar(
            scal[:, 0:1], pa_sum, inv_n, EPS,
            mybir.AluOpType.mult, mybir.AluOpType.max,
        )
        nc.vector.reciprocal(out=scal[:, 1:2], in_=scal[:, 0:1])
        pa_bc = pap.tile([P, 2], f32, tag="pa")
        nc.tensor.matmul(pa_bc, lhsT=ones_m, rhs=scal, start=True, stop=True)
        ab = const.tile([P, 2], f32)
        nc.scalar.copy(out=ab, in_=pa_bc)
        alpha_bc = ab[:, 0:1]
        invalpha_bc = ab[:, 1:2]

        # ---- quant sprint: wq[:,j,:] = fp16 clip(round(w/alpha), -1, 1)
        # split across ScalarE (magic pass, even j) / VectorE (odd j +
        # second clip) / GpSimd (first clip, half the tiles).
        for j in range(NK):
            if j in wtile:
                wj = wtile[j]
            else:
                wj = ws.tile([P, d_out], f32, tag="ws", name=f"wrr{j}")
                nc.sync.dma_start(out=wj, in_=wt_ap[j * P : (j + 1) * P, :])
            qm = qtmp.tile([P, d_out], f16, tag="qm", name=f"qm{j}")
            if j % 2 == 0:
                nc.scalar.activation(
                    out=qm, in_=wj, func=Id, scale=invalpha_bc, bias=m16b,
                )
            else:
                nc.vector.tensor_scalar(
                    qm, wj, invalpha_bc, M16,
                    mybir.AluOpType.mult, mybir.AluOpType.add,
                )
            # qc = max(qm - 1536, -1); wq = min(qc, 1)
            qc = qtmp.tile([P, d_out], f16, tag="qc", name=f"qc{j}", bufs=1)
            eng = nc.gpsimd if j % 2 == 0 else nc.vector
            eng.tensor_scalar(
                qc, qm, M16, -1.0,
                mybir.AluOpType.subtract, mybir.AluOpType.max,
            )
            nc.vector.tensor_scalar(
                wq[:, j, :], qc, 1.0, None, mybir.AluOpType.min,
            )

        # ---- main loop ----
        for i in range(NX):
            if i == 0 and NX > 1:
                staged[1] = x_stage(1)
            if i + 2 < NX:
                staged[i + 2] = x_stage(i + 2)
            beta, xqt = staged.pop(i)
            scale = small.tile([P, 1], f32, tag="scale", name=f"scale{i}")
            nc.vector.tensor_tensor(
                scale, beta, alpha_bc, mybir.AluOpType.mult
            )
            ysb = ybf.tile([P, d_out], f16, tag="ysb", name=f"ysb{i}")
            # both psum halves accumulate k-by-k so each wq[k] is consumed
            # for the whole tile as soon as it is quantized
            pys = [
                pyp.tile([P, 1024], f32, tag="py", name=f"py{i}_{h}")
                for h in range(2)
            ]
            for k in range(NK):
                for h in range(2):
                    for b in range(2):
                        nc.tensor.matmul(
                            pys[h][:, b * 512 : (b + 1) * 512],
                            lhsT=xqt[:, k, :],
                            rhs=wq[:, k, (h * 2 + b) * 512 : (h * 2 + b + 1) * 512],
                            start=(k == 0),
                            stop=(k == NK - 1),
                        )
            for h in range(2):
                nc.scalar.mul(
                    out=ysb[:, h * 1024 : (h + 1) * 1024], in_=pys[h],
                    mul=scale,
                )
            nc.sync.dma_start(out=y_ap[i * P : (i + 1) * P, :], in_=ysb)


def build_nc(d_in=D_IN, d_out=D_OUT, n_tok=TOK_PER_CORE, n_cores=N_CORES):
    nc = bass.Bass(
        "TRN2", target_bir_lowering=False, debug=False, num_devices=n_cores
    )
    x = nc.dram_tensor("x", [n_tok, d_in], mybir.dt.float32, kind="ExternalInput")
    wt = nc.dram_tensor("wt", [d_in, d_out], mybir.dt.float32, kind="ExternalInput")
    y = nc.dram_tensor("y", [n_tok, d_out], mybir.dt.float16, kind="ExternalOutput")
    with tile.TileContext(nc) as tc:
        emit_bitlinear(tc, y[:, :], x[:, :], wt[:, :], d_in, d_out, n_tok)
    _split_excess_waits(nc)
    return nc


_NC_CACHE = {}


def _run(x: np.ndarray, weight: np.ndarray, **spmd_kwargs):
    x = np.ascontiguousarray(np.asarray(x, dtype=np.float32))
    weight = np.asarray(weight, dtype=np.float32)
    b, s, d = x.shape
    n_tok_full = b * s
    n_tok = n_tok_full // N_CORES
    wt = np.ascontiguousarray(weight.T)

    key = (d, weight.shape[0], n_tok)
    if key not in _NC_CACHE:
        _NC_CACHE[key] = build_nc(d_in=d, d_out=weight.shape[0], n_tok=n_tok)
    nc = _NC_CACHE[key]

    x2d = x.reshape(n_tok_full, d)
    in_maps = [
        {"x": x2d[c * n_tok : (c + 1) * n_tok], "wt": wt} for c in range(N_CORES)
    ]
    res = run_bass_kernel_spmd(
        nc, in_maps, core_ids=list(range(N_CORES)), **spmd_kwargs
    )
    y = np.concatenate([res.results[c]["y"] for c in range(N_CORES)], axis=0)
    y = y.astype(np.float32)
    return y.reshape(b, s, weight.shape[0]), res


def kernel(x: np.ndarray, weight: np.ndarray) -> np.ndarray:
    y, _ = _run(x, weight)
    return y


# revision 26
# speedup vs baseline: 1.7121x; 1.7121x over previous
"""BitLinear (ternary weight / int8-activation quantized matmul) Trainium2 kernel.

Reference semantics (for x:(B,S,D), weight:(O,D)):
    alpha = max(mean(|W|), 1e-8)                     # per-tensor scalar
    w_q   = clip(round(W/alpha), -1, 1)              # ternary
    beta  = max(max|x| / 127, 1e-8)                  # per token
    x_q   = clip(round(x/beta), -127, 127)           # int8 range
    y     = (x_q @ w_q.T) * alpha * beta

Sharding: data-parallel over the 16384 tokens across 8 NeuronCores
(2048 tokens/core); full weight replicated per core (no collectives —
a 4-byte AllReduce measures ~56us on this runtime, slower than just
reading W locally).

Quantized GEMM runs in fp16 which is EXACT here: x_q in [-127,127] and
w_q in {-1,0,1} are exactly representable and fp32 PSUM accumulation
of the integer partial sums is exact.

Rounding uses the fp16 magic-number trick: (v + 1536.0) computed in
fp32 inside an engine and converted to fp16 on output rounds v to the
nearest integer (RNE), since ulp(1536)=1 in fp16. x quantization is
thus a SINGLE scalar-engine activation pass producing xm = 1536 + x_q;
the PE-transpose evacuation pass removes the offset. (The W path
removes its offset during the clip passes — keeping offset values in
the GEMM operands trips the PE's HAM power throttle and slows every
matmul by ~30%.)

Schedule: W tiles are DMA'd first (whole-tile DMAs spread descriptors
across all 16 queues; ~2.9us per 1 MiB tile of added completion
latency) with |W| accumulation riding the stream on ScalarE, so alpha
is ready right after the last W tile lands. The post-alpha
quantization sprint is split across ScalarE/VectorE/GpSimd, and the
matmul loop accumulates both PSUM halves k-by-k so the PE starts on
wq[0] immediately and trickle-consumes the rest.
"""

import numpy as np

import bass_rust
import concourse.bass as bass
import concourse.mybir as mybir
import concourse.tile as tile
from concourse.bass_utils import run_bass_kernel_spmd
from concourse.masks import make_identity

N_CORES = 8
P = 128
M16 = 1536.0  # 1.5 * 2**10 : fp16 RNE round-to-integer magic constant
EPS = 1e-8

# Full-problem shapes (hardcoded per the grading contract)
FULL_B, FULL_S, FULL_D = 4, 4096, 2048
D_IN = 2048
D_OUT = 2048
TOK_PER_CORE = FULL_B * FULL_S // N_CORES  # 2048

WS_BUFS = 9    # W f32 tiles kept resident through the alpha pass; the
               # remaining 7 stream through a small pool for |W| accum
               # and are re-read after alpha (SBUF cannot hold all 16
               # f32 tiles plus the fp16 wq)


def _split_excess_waits(nc, max_waits=1):
    """This container's walrus accepts at most `max_waits` sync waits per
    instruction; move excess waits onto preceding same-engine nops."""
    n = 0
    for f in nc.m.functions:
        for bb in f.blocks:
            insts = list(bb.instructions)
            out = []
            changed = False
            for inst in insts:
                si = inst.sync_info
                if si is not None and len(si.on_wait) > max_waits:
                    waits = list(si.on_wait)
                    extra, keep = waits[:-max_waits], waits[-max_waits:]
                    for i in range(0, len(extra), max_waits):
                        chunk = extra[i : i + max_waits]
                        n += 1
                        nop = mybir.InstNoOp(name=f"waitsplit-{n}")
                        nop.engine = inst.engine
                        nop.sync_info = bass_rust.SyncInfo(
                            on_wait=chunk, on_update=[]
                        )
                        out.append(nop)
                    inst.sync_info = bass_rust.SyncInfo(
                        on_wait=keep, on_update=list(si.on_update)
                    )
                    changed = True
                out.append(inst)
            if changed:
                bb.instructions = out


def emit_bitlinear(tc, y_ap, x_ap, wt_ap, d_in, d_out, n_tok):
    """Emit the per-core kernel body.

    x_ap:  [n_tok, d_in]  f32 token rows for this core
    wt_ap: [d_in, d_out]  f32 transposed weight (wt[i,o] = W[o,i])
    y_ap:  [n_tok, d_out] f16 output (host upcasts to f32)
    """
    from contextlib import ExitStack

    nc = tc.nc
    f32 = mybir.dt.float32
    f16 = mybir.dt.float16
    bf16 = mybir.dt.bfloat16
    NK = d_in // P          # 16 contraction tiles
    NX = n_tok // P         # 16 token tiles
    inv_n = 1.0 / float(d_in * d_out)
    Id = mybir.ActivationFunctionType.Identity
    Ab = mybir.ActivationFunctionType.Abs

    with ExitStack() as ctx:
        const = ctx.enter_context(tc.tile_pool(name="const", bufs=1))
        ws = ctx.enter_context(tc.tile_pool(name="ws", bufs=WS_BUFS))
        ws2 = ctx.enter_context(tc.tile_pool(name="ws2", bufs=2))
        wqp = ctx.enter_context(tc.tile_pool(name="wqp", bufs=1))
        qtmp = ctx.enter_context(tc.tile_pool(name="qtmp", bufs=2))
        xf32 = ctx.enter_context(tc.tile_pool(name="xf32", bufs=2))
        xqm = ctx.enter_context(tc.tile_pool(name="xqm", bufs=2))
        xqtp = ctx.enter_context(tc.tile_pool(name="xqtp", bufs=2))
        ybf = ctx.enter_context(tc.tile_pool(name="ybf", bufs=1))
        small = ctx.enter_context(tc.tile_pool(name="small", bufs=16))
        pyp = ctx.enter_context(tc.tile_pool(name="pyp", bufs=4, space="PSUM"))

        ones_k = const.tile([P, 1], f32)
        nc.vector.memset(ones_k, 1.0)
        ones_m = const.tile([1, P], f32)
        nc.vector.memset(ones_m, 1.0)
        m16b = const.tile([P, 1], f32)
        nc.vector.memset(m16b, M16)
        partials = const.tile([P, NK], f32)
        wq = wqp.tile([P, NK, d_out], f16)

        # ---- head DMAs: x0 (for staging), then the full W stream ----
        x_in = {}
        xi0 = xf32.tile([P, d_in], f32, tag="xi", name="xi0")
        nc.sync.dma_start(out=xi0, in_=x_ap[0:P, :])
        x_in[0] = xi0
        wtile = {}
        for j in range(NK):
            if j < WS_BUFS:
                wj = ws.tile([P, d_out], f32, tag="ws", name=f"w{j}")
                wtile[j] = wj
            else:
                wj = ws2.tile([P, d_out], f32, tag="ws2", name=f"w{j}")
            nc.sync.dma_start(out=wj, in_=wt_ap[j * P : (j + 1) * P, :])
            # |W| row-sums ride the stream on ScalarE
            trash = qtmp.tile([P, d_out], bf16, tag="qm", name=f"trash{j}")
            nc.scalar.activation(
                out=trash, in_=wj, func=Ab, accum_out=partials[:, j : j + 1],
            )

        def x_dma(i):
            if i in x_in:
                return x_in.pop(i)
            xi = xf32.tile([P, d_in], f32, tag="xi", name=f"xi{i}")
            nc.sync.dma_start(out=xi, in_=x_ap[i * P : (i + 1) * P, :])
            return xi

        def x_stage(i):
            xi = x_dma(i)
            am = small.tile([P, 1], f32, tag="am", name=f"am{i}")
            nc.vector.tensor_reduce(
                out=am, in_=xi, axis=mybir.AxisListType.X,
                op=mybir.AluOpType.max, apply_absolute_value=True,
            )
            beta = small.tile([P, 1], f32, tag="beta", name=f"beta{i}")
            nc.vector.tensor_scalar(
                beta, am, 1.0 / 127.0, EPS,
                mybir.AluOpType.mult, mybir.AluOpType.max,
            )
            invb = small.tile([P, 1], f32, tag="invb", name=f"invb{i}")
            nc.vector.reciprocal(out=invb, in_=beta)
            # xm = fp16(x*invb + 1536) == 1536 + x_q  (exact RNE round),
            # then remove the offset in place and transpose via the DMA
            # XBAR (keeps the PE free for matmuls).
            xm = xqm.tile([P, d_in], f16, tag="xm", name=f"xm{i}")
            nc.scalar.activation(out=xm, in_=xi, func=Id, scale=invb, bias=m16b)
            nc.vector.tensor_scalar(
                xm, xm, M16, None, mybir.AluOpType.subtract,
            )
            xqt = xqtp.tile([P, NK, P], f16, tag="xqt", name=f"xqt{i}")
            nc.sync.dma_start_transpose(out=xqt, in_=xm)
            return beta, xqt

        staged = {}
        staged[0] = x_stage(0)

        # ---- alpha = max(mean|W|, EPS); broadcast alpha & 1/alpha ----
        total = const.tile([P, 1], f32)
        nc.vector.tensor_reduce(
            out=total, in_=partials, axis=mybir.AxisListType.X,
            op=mybir.AluOpType.add,
        )
        pa_sum = pyp.tile([1, 1], f32, tag="py", name="pa_sum")
        nc.tensor.matmul(pa_sum, lhsT=total, rhs=ones_k, start=True, stop=True)
        scal = const.tile([1, 2], f32)
        nc.vector.tensor_scalar(
            scal[:, 0:1], pa_sum, inv_n, EPS,
            mybir.AluOpType.mult, mybir.AluOpType.max,
        )
        nc.vector.reciprocal(out=scal[:, 1:2], in_=scal[:, 0:1])
        pa_bc = pyp.tile([P, 2], f32, tag="py", name="pa_bc")
        nc.tensor.matmul(pa_bc, lhsT=ones_m, rhs=scal, start=True, stop=True)
        ab = const.tile([P, 2], f32)
        nc.scalar.copy(out=ab, in_=pa_bc)
        alpha_bc = ab[:, 0:1]
        invalpha_bc = ab[:, 1:2]

        # ---- quant sprint: wq[:,j,:] = fp16 clip(round(w/alpha), -1, 1)
        # split across ScalarE (magic pass, even j) / VectorE (odd j +
        # second clip) / GpSimd (first clip, half the tiles).
        for j in range(NK):
            if j in wtile:
                wj = wtile[j]
            else:
                wj = ws.tile([P, d_out], f32, tag="ws", name=f"wrr{j}")
                nc.sync.dma_start(out=wj, in_=wt_ap[j * P : (j + 1) * P, :])
            qm = qtmp.tile([P, d_out], f16, tag="qm", name=f"qm{j}")
            if j < 10:
                nc.scalar.activation(
                    out=qm, in_=wj, func=Id, scale=invalpha_bc, bias=m16b,
                )
            else:
                nc.vector.tensor_scalar(
                    qm, wj, invalpha_bc, M16,
                    mybir.AluOpType.mult, mybir.AluOpType.add,
                )
            # qc = max(qm - 1536, -1); wq = min(qc, 1)
            qc = qtmp.tile([P, d_out], f16, tag="qc", name=f"qc{j}", bufs=1)
            nc.vector.tensor_scalar(
                qc, qm, M16, -1.0,
                mybir.AluOpType.subtract, mybir.AluOpType.max,
            )
            nc.vector.tensor_scalar(
                wq[:, j, :], qc, 1.0, None, mybir.AluOpType.min,
            )

        # ---- main loop ----
        for i in range(NX):
            if i == 0 and NX > 1:
                staged[1] = x_stage(1)
            if i + 2 < NX:
                staged[i + 2] = x_stage(i + 2)
            beta, xqt = staged.pop(i)
            scale = small.tile([P, 1], f32, tag="scale", name=f"scale{i}")
            nc.vector.tensor_tensor(
                scale, beta, alpha_bc, mybir.AluOpType.mult
            )
            ysb = ybf.tile([P, d_out], f16, tag="ysb", name=f"ysb{i}")
            # both psum halves accumulate k-by-k so each wq[k] is consumed
            # for the whole tile as soon as it is quantized
            pys = [
                pyp.tile([P, 1024], f32, tag="py", name=f"py{i}_{h}")
                for h in range(2)
            ]
            for k in range(NK):
                for h in range(2):
                    for b in range(2):
                        nc.tensor.matmul(
                            pys[h][:, b * 512 : (b + 1) * 512],
                            lhsT=xqt[:, k, :],
                            rhs=wq[:, k, (h * 2 + b) * 512 : (h * 2 + b + 1) * 512],
                            start=(k == 0),
                            stop=(k == NK - 1),
                        )
            for h in range(2):
                nc.scalar.mul(
                    out=ysb[:, h * 1024 : (h + 1) * 1024], in_=pys[h],
                    mul=scale,
                )
            nc.sync.dma_start(out=y_ap[i * P : (i + 1) * P, :], in_=ysb)


def build_nc(d_in=D_IN, d_out=D_OUT, n_tok=TOK_PER_CORE, n_cores=N_CORES):
    nc = bass.Bass(
        "TRN2", target_bir_lowering=False, debug=False, num_devices=n_cores
    )
    x = nc.dram_tensor("x", [n_tok, d_in], mybir.dt.float32, kind="ExternalInput")
    wt = nc.dram_tensor("wt", [d_in, d_out], mybir.dt.float32, kind="ExternalInput")
    y = nc.dram_tensor("y", [n_tok, d_out], mybir.dt.float16, kind="ExternalOutput")
    with tile.TileContext(nc) as tc:
        emit_bitlinear(tc, y[:, :], x[:, :], wt[:, :], d_in, d_out, n_tok)
    _split_excess_waits(nc)
    return nc


_NC_CACHE = {}


def _run(x: np.ndarray, weight: np.ndarray, **spmd_kwargs):
    x = np.ascontiguousarray(np.asarray(x, dtype=np.float32))
    weight = np.asarray(weight, dtype=np.float32)
    b, s, d = x.shape
    n_tok_full = b * s
    n_tok = n_tok_full // N_CORES
    wt = np.ascontiguousarray(weight.T)

    key = (d, weight.shape[0], n_tok)
    if key not in _NC_CACHE:
        _NC_CACHE[key] = build_nc(d_in=d, d_out=weight.shape[0], n_tok=n_tok)
    nc = _NC_CACHE[key]

    x2d = x.reshape(n_tok_full, d)
    in_maps = [
        {"x": x2d[c * n_tok : (c + 1) * n_tok], "wt": wt} for c in range(N_CORES)
    ]
    res = run_bass_kernel_spmd(
        nc, in_maps, core_ids=list(range(N_CORES)), **spmd_kwargs
    )
    y = np.concatenate([res.results[c]["y"] for c in range(N_CORES)], axis=0)
    y = y.astype(np.float32)
    return y.reshape(b, s, weight.shape[0]), res


def kernel(x: np.ndarray, weight: np.ndarray) -> np.ndarray:
    y, _ = _run(x, weight)
    return y


# revision 34
# speedup vs baseline: 1.7200x; 1.0046x over previous
"""BitLinear (ternary weight / int8-activation quantized matmul) Trainium2 kernel.

Reference semantics (for x:(B,S,D), weight:(O,D)):
    alpha = max(mean(|W|), 1e-8)                     # per-tensor scalar
    w_q   = clip(round(W/alpha), -1, 1)              # ternary
    beta  = max(max|x| / 127, 1e-8)                  # per token
    x_q   = clip(round(x/beta), -127, 127)           # int8 range
    y     = (x_q @ w_q.T) * alpha * beta

Sharding: data-parallel over the 16384 tokens across 8 NeuronCores
(2048 tokens/core); full weight replicated per core (no collectives —
a 4-byte AllReduce measures ~56us on this runtime, slower than just
reading W locally).

Quantized GEMM runs in fp16 which is EXACT here: x_q in [-127,127] and
w_q in {-1,0,1} are exactly representable and fp32 PSUM accumulation
of the integer partial sums is exact.

Rounding uses the fp16 magic-number trick: (v + 1536.0) computed in
fp32 inside an engine and converted to fp16 on output rounds v to the
nearest integer (RNE), since ulp(1536)=1 in fp16. x quantization is
thus a SINGLE scalar-engine activation pass producing xm = 1536 + x_q;
the PE-transpose evacuation pass removes the offset. (The W path
removes its offset during the clip passes — keeping offset values in
the GEMM operands trips the PE's HAM power throttle and slows every
matmul by ~30%.)

Schedule: W tiles are DMA'd first (whole-tile DMAs spread descriptors
across all 16 queues; ~2.9us per 1 MiB tile of added completion
latency) with |W| accumulation riding the stream on ScalarE, so alpha
is ready right after the last W tile lands. The post-alpha
quantization sprint is split across ScalarE/VectorE/GpSimd, and the
matmul loop accumulates both PSUM halves k-by-k so the PE starts on
wq[0] immediately and trickle-consumes the rest.
"""

import numpy as np

import bass_rust
import concourse.bass as bass
import concourse.mybir as mybir
import concourse.tile as tile
from concourse.bass_utils import run_bass_kernel_spmd
from concourse.masks import make_identity

N_CORES = 8
P = 128
M16 = 1536.0  # 1.5 * 2**10 : fp16 RNE round-to-integer magic constant
EPS = 1e-8

# Full-problem shapes (hardcoded per the grading contract)
FULL_B, FULL_S, FULL_D = 4, 4096, 2048
D_IN = 2048
D_OUT = 2048
TOK_PER_CORE = FULL_B * FULL_S // N_CORES  # 2048

WS_BUFS = 9    # W f32 tiles kept resident through the alpha pass; the
               # remaining 7 stream through a small pool for |W| accum
               # and are re-read after alpha (SBUF cannot hold all 16
               # f32 tiles plus the fp16 wq)


def _split_excess_waits(nc, max_waits=1):
    """This container's walrus accepts at most `max_waits` sync waits per
    instruction; move excess waits onto preceding same-engine nops."""
    n = 0
    for f in nc.m.functions:
        for bb in f.blocks:
            insts = list(bb.instructions)
            out = []
            changed = False
            for inst in insts:
                si = inst.sync_info
                if si is not None and len(si.on_wait) > max_waits:
                    waits = list(si.on_wait)
                    extra, keep = waits[:-max_waits], waits[-max_waits:]
                    for i in range(0, len(extra), max_waits):
                        chunk = extra[i : i + max_waits]
                        n += 1
                        nop = mybir.InstNoOp(name=f"waitsplit-{n}")
                        nop.engine = inst.engine
                        nop.sync_info = bass_rust.SyncInfo(
                            on_wait=chunk, on_update=[]
                        )
                        out.append(nop)
                    inst.sync_info = bass_rust.SyncInfo(
                        on_wait=keep, on_update=list(si.on_update)
                    )
                    changed = True
                out.append(inst)
            if changed:
                bb.instructions = out


def emit_bitlinear(tc, y_ap, x_ap, wt_ap, d_in, d_out, n_tok):
    """Emit the per-core kernel body.

    x_ap:  [n_tok, d_in]  f32 token rows for this core
    wt_ap: [d_in, d_out]  f32 transposed weight (wt[i,o] = W[o,i])
    y_ap:  [n_tok, d_out] f16 output (host upcasts to f32)
    """
    from contextlib import ExitStack

    nc = tc.nc
    f32 = mybir.dt.float32
    f16 = mybir.dt.float16
    bf16 = mybir.dt.bfloat16
    NK = d_in // P          # 16 contraction tiles
    NX = n_tok // P         # 16 token tiles
    inv_n = 1.0 / float(d_in * d_out)
    Id = mybir.ActivationFunctionType.Identity
    Ab = mybir.ActivationFunctionType.Abs

    with ExitStack() as ctx:
        const = ctx.enter_context(tc.tile_pool(name="const", bufs=1))
        ws = ctx.enter_context(tc.tile_pool(name="ws", bufs=WS_BUFS))
        ws2 = ctx.enter_context(tc.tile_pool(name="ws2", bufs=2))
        wqp = ctx.enter_context(tc.tile_pool(name="wqp", bufs=1))
        qtmp = ctx.enter_context(tc.tile_pool(name="qtmp", bufs=2))
        xf32 = ctx.enter_context(tc.tile_pool(name="xf32", bufs=2))
        xqm = ctx.enter_context(tc.tile_pool(name="xqm", bufs=2))
        xqtp = ctx.enter_context(tc.tile_pool(name="xqtp", bufs=3))
        ybf = ctx.enter_context(tc.tile_pool(name="ybf", bufs=1))
        small = ctx.enter_context(tc.tile_pool(name="small", bufs=16))
        pyp = ctx.enter_context(tc.tile_pool(name="pyp", bufs=4, space="PSUM"))

        ones_k = const.tile([P, 1], f32)
        nc.vector.memset(ones_k, 1.0)
        ones_m = const.tile([1, P], f32)
        nc.vector.memset(ones_m, 1.0)
        m16b = const.tile([P, 1], f32)
        nc.vector.memset(m16b, M16)
        partials = const.tile([P, NK], f32)
        wq = wqp.tile([P, NK, d_out], f16)

        # ---- head DMAs: the full W stream first (alpha gates everything)
        x_in = {}
        wtile = {}
        for j in range(NK):
            if j < WS_BUFS:
                wj = ws.tile([P, d_out], f32, tag="ws", name=f"w{j}")
                wtile[j] = wj
            else:
                wj = ws2.tile([P, d_out], f32, tag="ws2", name=f"w{j}")
            # alternate the issuing HWDGE engine so descriptor-ring refill
            # isn't serialized on one sequencer
            dma_eng = nc.sync if j % 2 == 0 else nc.scalar
            dma_eng.dma_start(out=wj, in_=wt_ap[j * P : (j + 1) * P, :])
            # |W| row-sums ride the stream on ScalarE (2.2us/tile, paced
            # well under the ~2.9us/tile DMA arrival rate)
            trash = qtmp.tile([P, d_out], bf16, tag="qm", name=f"trash{j}")
            nc.scalar.activation(
                out=trash, in_=wj, func=Ab,
                accum_out=partials[:, j : j + 1],
            )
            if j == 13:
                # x0 lands just before the W tail so its staging chain
                # finishes right as alpha resolves
                xi0 = xf32.tile([P, d_in], f32, tag="xi", name="xi0")
                nc.sync.dma_start(out=xi0, in_=x_ap[0:P, :])
                x_in[0] = xi0

        def x_dma(i):
            if i in x_in:
                return x_in.pop(i)
            xi = xf32.tile([P, d_in], f32, tag="xi", name=f"xi{i}")
            nc.sync.dma_start(out=xi, in_=x_ap[i * P : (i + 1) * P, :])
            return xi

        def x_stage(i):
            xi = x_dma(i)
            am = small.tile([P, 1], f32, tag="am", name=f"am{i}")
            nc.vector.tensor_reduce(
                out=am, in_=xi, axis=mybir.AxisListType.X,
                op=mybir.AluOpType.max, apply_absolute_value=True,
            )
            beta = small.tile([P, 1], f32, tag="beta", name=f"beta{i}")
            nc.vector.tensor_scalar(
                beta, am, 1.0 / 127.0, EPS,
                mybir.AluOpType.mult, mybir.AluOpType.max,
            )
            invb = small.tile([P, 1], f32, tag="invb", name=f"invb{i}")
            nc.vector.reciprocal(out=invb, in_=beta)
            # xm = fp16(x*invb + 1536) == 1536 + x_q  (exact RNE round),
            # then remove the offset in place and transpose via the DMA
            # XBAR (keeps the PE free for matmuls).
            xm = xqm.tile([P, d_in], f16, tag="xm", name=f"xm{i}")
            nc.scalar.activation(out=xm, in_=xi, func=Id, scale=invb, bias=m16b)
            nc.vector.tensor_scalar(
                xm, xm, M16, None, mybir.AluOpType.subtract,
            )
            xqt = xqtp.tile([P, NK, P], f16, tag="xqt", name=f"xqt{i}")
            nc.sync.dma_start_transpose(out=xqt, in_=xm)
            return beta, xqt

        staged = {}
        staged[0] = x_stage(0)

        # ---- alpha = max(mean|W|, EPS); broadcast alpha & 1/alpha ----
        total = const.tile([P, 1], f32)
        nc.vector.tensor_reduce(
            out=total, in_=partials, axis=mybir.AxisListType.X,
            op=mybir.AluOpType.add,
        )
        pa_sum = pyp.tile([1, 1], f32, tag="py", name="pa_sum")
        nc.tensor.matmul(pa_sum, lhsT=total, rhs=ones_k, start=True, stop=True)
        scal = const.tile([1, 2], f32)
        nc.vector.tensor_scalar(
            scal[:, 0:1], pa_sum, inv_n, EPS,
            mybir.AluOpType.mult, mybir.AluOpType.max,
        )
        nc.vector.reciprocal(out=scal[:, 1:2], in_=scal[:, 0:1])
        pa_bc = pyp.tile([P, 2], f32, tag="py", name="pa_bc")
        nc.tensor.matmul(pa_bc, lhsT=ones_m, rhs=scal, start=True, stop=True)
        ab = const.tile([P, 2], f32)
        nc.scalar.copy(out=ab, in_=pa_bc)
        alpha_bc = ab[:, 0:1]
        invalpha_bc = ab[:, 1:2]

        # ---- quant sprint: wq[:,j,:] = fp16 clip(round(w/alpha), -1, 1)
        # split across ScalarE (magic pass, even j) / VectorE (odd j +
        # second clip) / GpSimd (first clip, half the tiles).
        for j in range(NK):
            if j in wtile:
                wj = wtile[j]
            else:
                wj = ws.tile([P, d_out], f32, tag="ws", name=f"wrr{j}")
                nc.sync.dma_start(out=wj, in_=wt_ap[j * P : (j + 1) * P, :])
            qm = qtmp.tile([P, d_out], f16, tag="qm", name=f"qm{j}")
            if j < 10:
                nc.scalar.activation(
                    out=qm, in_=wj, func=Id, scale=invalpha_bc, bias=m16b,
                )
            else:
                nc.vector.tensor_scalar(
                    qm, wj, invalpha_bc, M16,
                    mybir.AluOpType.mult, mybir.AluOpType.add,
                )
            # qc = max(qm - 1536, -1); wq = min(qc, 1)
            qc = qtmp.tile([P, d_out], f16, tag="qc", name=f"qc{j}", bufs=1)
            nc.vector.tensor_scalar(
                qc, qm, M16, -1.0,
                mybir.AluOpType.subtract, mybir.AluOpType.max,
            )
            nc.vector.tensor_scalar(
                wq[:, j, :], qc, 1.0, None, mybir.AluOpType.min,
            )

        # ---- main loop ----
        def tile_head(i):
            beta, xqt = staged.pop(i)
            scale = small.tile([P, 1], f32, tag="scale", name=f"scale{i}")
            nc.vector.tensor_tensor(
                scale, beta, alpha_bc, mybir.AluOpType.mult
            )
            pys = [
                pyp.tile([P, 1024], f32, tag="py", name=f"py{i}_{h}")
                for h in range(2)
            ]
            return xqt, scale, pys

        def mm(xqt, pys, k):
            # both psum halves accumulate k-by-k so each wq[k] is consumed
            # for the whole tile as soon as it is quantized
            for h in range(2):
                for b in range(2):
                    nc.tensor.matmul(
                        pys[h][:, b * 512 : (b + 1) * 512],
                        lhsT=xqt[:, k, :],
                        rhs=wq[:, k, (h * 2 + b) * 512 : (h * 2 + b + 1) * 512],
                        start=(k == 0),
                        stop=(k == NK - 1),
                    )

        def tile_tail(i, scale, pys):
            ysb = ybf.tile([P, d_out], f16, tag="ysb", name=f"ysb{i}")
            for h in range(2):
                nc.scalar.mul(
                    out=ysb[:, h * 1024 : (h + 1) * 1024], in_=pys[h],
                    mul=scale,
                )
            nc.sync.dma_start(out=y_ap[i * P : (i + 1) * P, :], in_=ysb)

        start_i = 0
        if NX >= 2:
            # fuse the first two tiles: all 8 PSUM banks in flight, k-steps
            # interleaved, so the W re-read/quant trickle is fully covered
            staged[1] = x_stage(1)
            if NX > 2:
                staged[2] = x_stage(2)
            t0 = tile_head(0)
            t1 = tile_head(1)
            for k in range(NK):
                mm(t0[0], t0[2], k)
                mm(t1[0], t1[2], k)
            tile_tail(0, t0[1], t0[2])
            tile_tail(1, t1[1], t1[2])
            start_i = 2
        for i in range(start_i, NX):
            if i == 2:
                for ii in (3, 4):
                    if ii < NX:
                        staged[ii] = x_stage(ii)
            elif i + 2 < NX:
                staged[i + 2] = x_stage(i + 2)
            xqt, scale, pys = tile_head(i)
            for k in range(NK):
                mm(xqt, pys, k)
            tile_tail(i, scale, pys)


def build_nc(d_in=D_IN, d_out=D_OUT, n_tok=TOK_PER_CORE, n_cores=N_CORES):
    nc = bass.Bass(
        "TRN2", target_bir_lowering=False, debug=False, num_devices=n_cores
    )
    x = nc.dram_tensor("x", [n_tok, d_in], mybir.dt.float32, kind="ExternalInput")
    wt = nc.dram_tensor("wt", [d_in, d_out], mybir.dt.float32, kind="ExternalInput")
    y = nc.dram_tensor("y", [n_tok, d_out], mybir.dt.float16, kind="ExternalOutput")
    with tile.TileContext(nc) as tc:
        emit_bitlinear(tc, y[:, :], x[:, :], wt[:, :], d_in, d_out, n_tok)
    _split_excess_waits(nc)
    return nc


_NC_CACHE = {}


def _run(x: np.ndarray, weight: np.ndarray, **spmd_kwargs):
    x = np.ascontiguousarray(np.asarray(x, dtype=np.float32))
    weight = np.asarray(weight, dtype=np.float32)
    b, s, d = x.shape
    n_tok_full = b * s
    n_tok = n_tok_full // N_CORES
    wt = np.ascontiguousarray(weight.T)

    key = (d, weight.shape[0], n_tok)
    if key not in _NC_CACHE:
        _NC_CACHE[key] = build_nc(d_in=d, d_out=weight.shape[0], n_tok=n_tok)
    nc = _NC_CACHE[key]

    x2d = x.reshape(n_tok_full, d)
    in_maps = [
        {"x": x2d[c * n_tok : (c + 1) * n_tok], "wt": wt} for c in range(N_CORES)
    ]
    res = run_bass_kernel_spmd(
        nc, in_maps, core_ids=list(range(N_CORES)), **spmd_kwargs
    )
    y = np.concatenate([res.results[c]["y"] for c in range(N_CORES)], axis=0)
    y = y.astype(np.float32)
    return y.reshape(b, s, weight.shape[0]), res


def kernel(x: np.ndarray, weight: np.ndarray) -> np.ndarray:
    y, _ = _run(x, weight)
    return y


# revision 35
# speedup vs baseline: 1.7333x; 1.0077x over previous
"""BitLinear (ternary weight / int8-activation quantized matmul) Trainium2 kernel.

Reference semantics (for x:(B,S,D), weight:(O,D)):
    alpha = max(mean(|W|), 1e-8)                     # per-tensor scalar
    w_q   = clip(round(W/alpha), -1, 1)              # ternary
    beta  = max(max|x| / 127, 1e-8)                  # per token
    x_q   = clip(round(x/beta), -127, 127)           # int8 range
    y     = (x_q @ w_q.T) * alpha * beta

Sharding: data-parallel over the 16384 tokens across 8 NeuronCores
(2048 tokens/core); full weight replicated per core (no collectives —
a 4-byte AllReduce measures ~56us on this runtime, slower than just
reading W locally).

Quantized GEMM runs in fp16 which is EXACT here: x_q in [-127,127] and
w_q in {-1,0,1} are exactly representable and fp32 PSUM accumulation
of the integer partial sums is exact.

Rounding uses the fp16 magic-number trick: (v + 1536.0) computed in
fp32 inside an engine and converted to fp16 on output rounds v to the
nearest integer (RNE), since ulp(1536)=1 in fp16. x quantization is
thus a SINGLE scalar-engine activation pass producing xm = 1536 + x_q;
the PE-transpose evacuation pass removes the offset. (The W path
removes its offset during the clip passes — keeping offset values in
the GEMM operands trips the PE's HAM power throttle and slows every
matmul by ~30%.)

Schedule: W tiles are DMA'd first (whole-tile DMAs spread descriptors
across all 16 queues; ~2.9us per 1 MiB tile of added completion
latency) with |W| accumulation riding the stream on ScalarE, so alpha
is ready right after the last W tile lands. The post-alpha
quantization sprint is split across ScalarE/VectorE/GpSimd, and the
matmul loop accumulates both PSUM halves k-by-k so the PE starts on
wq[0] immediately and trickle-consumes the rest.
"""

import numpy as np

import bass_rust
import concourse.bass as bass
import concourse.mybir as mybir
import concourse.tile as tile
from concourse.bass_utils import run_bass_kernel_spmd
from concourse.masks import make_identity

N_CORES = 8
P = 128
M16 = 1536.0  # 1.5 * 2**10 : fp16 RNE round-to-integer magic constant
EPS = 1e-8

# Full-problem shapes (hardcoded per the grading contract)
FULL_B, FULL_S, FULL_D = 4, 4096, 2048
D_IN = 2048
D_OUT = 2048
TOK_PER_CORE = FULL_B * FULL_S // N_CORES  # 2048

WS_BUFS = 9    # W f32 tiles kept resident through the alpha pass; the
               # remaining 7 stream through a small pool for |W| accum
               # and are re-read after alpha (SBUF cannot hold all 16
               # f32 tiles plus the fp16 wq)


def _split_excess_waits(nc, max_waits=1):
    """This container's walrus accepts at most `max_waits` sync waits per
    instruction; move excess waits onto preceding same-engine nops."""
    n = 0
    for f in nc.m.functions:
        for bb in f.blocks:
            insts = list(bb.instructions)
            out = []
            changed = False
            for inst in insts:
                si = inst.sync_info
                if si is not None and len(si.on_wait) > max_waits:
                    waits = list(si.on_wait)
                    extra, keep = waits[:-max_waits], waits[-max_waits:]
                    for i in range(0, len(extra), max_waits):
                        chunk = extra[i : i + max_waits]
                        n += 1
                        nop = mybir.InstNoOp(name=f"waitsplit-{n}")
                        nop.engine = inst.engine
                        nop.sync_info = bass_rust.SyncInfo(
                            on_wait=chunk, on_update=[]
                        )
                        out.append(nop)
                    inst.sync_info = bass_rust.SyncInfo(
                        on_wait=keep, on_update=list(si.on_update)
                    )
                    changed = True
                out.append(inst)
            if changed:
                bb.instructions = out


def emit_bitlinear(tc, y_ap, x_ap, wt_ap, d_in, d_out, n_tok):
    """Emit the per-core kernel body.

    x_ap:  [n_tok, d_in]  f32 token rows for this core
    wt_ap: [d_in, d_out]  f32 transposed weight (wt[i,o] = W[o,i])
    y_ap:  [n_tok, d_out] f16 output (host upcasts to f32)
    """
    from contextlib import ExitStack

    nc = tc.nc
    f32 = mybir.dt.float32
    f16 = mybir.dt.float16
    bf16 = mybir.dt.bfloat16
    NK = d_in // P          # 16 contraction tiles
    NX = n_tok // P         # 16 token tiles
    inv_n = 1.0 / float(d_in * d_out)
    Id = mybir.ActivationFunctionType.Identity
    Ab = mybir.ActivationFunctionType.Abs

    with ExitStack() as ctx:
        const = ctx.enter_context(tc.tile_pool(name="const", bufs=1))
        ws = ctx.enter_context(tc.tile_pool(name="ws", bufs=WS_BUFS))
        ws2 = ctx.enter_context(tc.tile_pool(name="ws2", bufs=2))
        wqp = ctx.enter_context(tc.tile_pool(name="wqp", bufs=1))
        qtmp = ctx.enter_context(tc.tile_pool(name="qtmp", bufs=2))
        xf32 = ctx.enter_context(tc.tile_pool(name="xf32", bufs=2))
        xqm = ctx.enter_context(tc.tile_pool(name="xqm", bufs=2))
        xqtp = ctx.enter_context(tc.tile_pool(name="xqtp", bufs=3))
        ybf = ctx.enter_context(tc.tile_pool(name="ybf", bufs=1))
        small = ctx.enter_context(tc.tile_pool(name="small", bufs=16))
        pyp = ctx.enter_context(tc.tile_pool(name="pyp", bufs=4, space="PSUM"))

        ones_k = const.tile([P, 1], f32)
        nc.vector.memset(ones_k, 1.0)
        ones_m = const.tile([1, P], f32)
        nc.vector.memset(ones_m, 1.0)
        m16b = const.tile([P, 1], f32)
        nc.vector.memset(m16b, M16)
        partials = const.tile([P, NK], f32)
        wq = wqp.tile([P, NK, d_out], f16)

        # ---- head DMAs: the full W stream first (alpha gates everything)
        x_in = {}
        wtile = {}
        for j in range(NK):
            if j < WS_BUFS:
                wj = ws.tile([P, d_out], f32, tag="ws", name=f"w{j}")
                wtile[j] = wj
            else:
                wj = ws2.tile([P, d_out], f32, tag="ws2", name=f"w{j}")
            nc.sync.dma_start(out=wj, in_=wt_ap[j * P : (j + 1) * P, :])
            # |W| row-sums ride the stream on ScalarE (2.2us/tile, paced
            # well under the ~2.9us/tile DMA arrival rate)
            trash = qtmp.tile([P, d_out], bf16, tag="qm", name=f"trash{j}")
            nc.scalar.activation(
                out=trash, in_=wj, func=Ab,
                accum_out=partials[:, j : j + 1],
            )
            if j == 13:
                # x0 lands just before the W tail so its staging chain
                # finishes right as alpha resolves
                xi0 = xf32.tile([P, d_in], f32, tag="xi", name="xi0")
                nc.sync.dma_start(out=xi0, in_=x_ap[0:P, :])
                x_in[0] = xi0

        def x_dma(i):
            if i in x_in:
                return x_in.pop(i)
            xi = xf32.tile([P, d_in], f32, tag="xi", name=f"xi{i}")
            nc.sync.dma_start(out=xi, in_=x_ap[i * P : (i + 1) * P, :])
            return xi

        def x_stage(i):
            xi = x_dma(i)
            am = small.tile([P, 1], f32, tag="am", name=f"am{i}")
            nc.vector.tensor_reduce(
                out=am, in_=xi, axis=mybir.AxisListType.X,
                op=mybir.AluOpType.max, apply_absolute_value=True,
            )
            beta = small.tile([P, 1], f32, tag="beta", name=f"beta{i}")
            nc.vector.tensor_scalar(
                beta, am, 1.0 / 127.0, EPS,
                mybir.AluOpType.mult, mybir.AluOpType.max,
            )
            invb = small.tile([P, 1], f32, tag="invb", name=f"invb{i}")
            nc.vector.reciprocal(out=invb, in_=beta)
            # xm = fp16(x*invb + 1536) == 1536 + x_q  (exact RNE round),
            # then remove the offset in place and transpose via the DMA
            # XBAR (keeps the PE free for matmuls).
            xm = xqm.tile([P, d_in], f16, tag="xm", name=f"xm{i}")
            nc.scalar.activation(out=xm, in_=xi, func=Id, scale=invb, bias=m16b)
            nc.vector.tensor_scalar(
                xm, xm, M16, None, mybir.AluOpType.subtract,
            )
            xqt = xqtp.tile([P, NK, P], f16, tag="xqt", name=f"xqt{i}")
            nc.sync.dma_start_transpose(out=xqt, in_=xm)
            return beta, xqt

        staged = {}
        staged[0] = x_stage(0)

        # ---- alpha = max(mean|W|, EPS); broadcast alpha & 1/alpha ----
        total = const.tile([P, 1], f32)
        nc.vector.tensor_reduce(
            out=total, in_=partials, axis=mybir.AxisListType.X,
            op=mybir.AluOpType.add,
        )
        pa_sum = pyp.tile([1, 1], f32, tag="py", name="pa_sum")
        nc.tensor.matmul(pa_sum, lhsT=total, rhs=ones_k, start=True, stop=True)
        scal = const.tile([1, 2], f32)
        nc.vector.tensor_scalar(
            scal[:, 0:1], pa_sum, inv_n, EPS,
            mybir.AluOpType.mult, mybir.AluOpType.max,
        )
        nc.vector.reciprocal(out=scal[:, 1:2], in_=scal[:, 0:1])
        pa_bc = pyp.tile([P, 2], f32, tag="py", name="pa_bc")
        nc.tensor.matmul(pa_bc, lhsT=ones_m, rhs=scal, start=True, stop=True)
        ab = const.tile([P, 2], f32)
        nc.scalar.copy(out=ab, in_=pa_bc)
        alpha_bc = ab[:, 0:1]
        invalpha_bc = ab[:, 1:2]

        # ---- quant sprint: wq[:,j,:] = fp16 clip(round(w/alpha), -1, 1)
        # split across ScalarE (magic pass, even j) / VectorE (odd j +
        # second clip) / GpSimd (first clip, half the tiles).
        for j in range(NK):
            if j in wtile:
                wj = wtile[j]
            else:
                wj = ws.tile([P, d_out], f32, tag="ws", name=f"wrr{j}")
                nc.sync.dma_start(out=wj, in_=wt_ap[j * P : (j + 1) * P, :])
            qm = qtmp.tile([P, d_out], f16, tag="qm", name=f"qm{j}")
            if j < 10:
                nc.scalar.activation(
                    out=qm, in_=wj, func=Id, scale=invalpha_bc, bias=m16b,
                )
            else:
                nc.vector.tensor_scalar(
                    qm, wj, invalpha_bc, M16,
                    mybir.AluOpType.mult, mybir.AluOpType.add,
                )
            # qc = max(qm - 1536, -1); wq = min(qc, 1)
            qc = qtmp.tile([P, d_out], f16, tag="qc", name=f"qc{j}", bufs=1)
            nc.vector.tensor_scalar(
                qc, qm, M16, -1.0,
                mybir.AluOpType.subtract, mybir.AluOpType.max,
            )
            nc.vector.tensor_scalar(
                wq[:, j, :], qc, 1.0, None, mybir.AluOpType.min,
            )

        # ---- main loop ----
        def tile_head(i):
            beta, xqt = staged.pop(i)
            scale = small.tile([P, 1], f32, tag="scale", name=f"scale{i}")
            nc.vector.tensor_tensor(
                scale, beta, alpha_bc, mybir.AluOpType.mult
            )
            pys = [
                pyp.tile([P, 1024], f32, tag="py", name=f"py{i}_{h}")
                for h in range(2)
            ]
            return xqt, scale, pys

        def mm(xqt, pys, k):
            # both psum halves accumulate k-by-k so each wq[k] is consumed
            # for the whole tile as soon as it is quantized
            for h in range(2):
                for b in range(2):
                    nc.tensor.matmul(
                        pys[h][:, b * 512 : (b + 1) * 512],
                        lhsT=xqt[:, k, :],
                        rhs=wq[:, k, (h * 2 + b) * 512 : (h * 2 + b + 1) * 512],
                        start=(k == 0),
                        stop=(k == NK - 1),
                    )

        def tile_tail(i, scale, pys):
            ysb = ybf.tile([P, d_out], f16, tag="ysb", name=f"ysb{i}")
            for h in range(2):
                nc.scalar.mul(
                    out=ysb[:, h * 1024 : (h + 1) * 1024], in_=pys[h],
                    mul=scale,
                )
            nc.sync.dma_start(out=y_ap[i * P : (i + 1) * P, :], in_=ysb)

        start_i = 0
        if NX >= 2:
            # fuse the first two tiles: all 8 PSUM banks in flight, k-steps
            # interleaved, so the W re-read/quant trickle is fully covered
            staged[1] = x_stage(1)
            if NX > 2:
                staged[2] = x_stage(2)
            t0 = tile_head(0)
            t1 = tile_head(1)
            for k in range(NK):
                mm(t0[0], t0[2], k)
                mm(t1[0], t1[2], k)
            tile_tail(0, t0[1], t0[2])
            tile_tail(1, t1[1], t1[2])
            start_i = 2
        for i in range(start_i, NX):
            if i == 2:
                for ii in (3, 4):
                    if ii < NX:
                        staged[ii] = x_stage(ii)
            elif i + 2 < NX:
                staged[i + 2] = x_stage(i + 2)
            xqt, scale, pys = tile_head(i)
            for k in range(NK):
                mm(xqt, pys, k)
            tile_tail(i, scale, pys)


def build_nc(d_in=D_IN, d_out=D_OUT, n_tok=TOK_PER_CORE, n_cores=N_CORES):
    nc = bass.Bass(
        "TRN2", target_bir_lowering=False, debug=False, num_devices=n_cores
    )
    x = nc.dram_tensor("x", [n_tok, d_in], mybir.dt.float32, kind="ExternalInput")
    wt = nc.dram_tensor("wt", [d_in, d_out], mybir.dt.float32, kind="ExternalInput")
    y = nc.dram_tensor("y", [n_tok, d_out], mybir.dt.float16, kind="ExternalOutput")
    with tile.TileContext(nc) as tc:
        emit_bitlinear(tc, y[:, :], x[:, :], wt[:, :], d_in, d_out, n_tok)
    _split_excess_waits(nc)
    return nc


_NC_CACHE = {}


def _run(x: np.ndarray, weight: np.ndarray, **spmd_kwargs):
    x = np.ascontiguousarray(np.asarray(x, dtype=np.float32))
    weight = np.asarray(weight, dtype=np.float32)
    b, s, d = x.shape
    n_tok_full = b * s
    n_tok = n_tok_full // N_CORES
    wt = np.ascontiguousarray(weight.T)

    key = (d, weight.shape[0], n_tok)
    if key not in _NC_CACHE:
        _NC_CACHE[key] = build_nc(d_in=d, d_out=weight.shape[0], n_tok=n_tok)
    nc = _NC_CACHE[key]

    x2d = x.reshape(n_tok_full, d)
    in_maps = [
        {"x": x2d[c * n_tok : (c + 1) * n_tok], "wt": wt} for c in range(N_CORES)
    ]
    res = run_bass_kernel_spmd(
        nc, in_maps, core_ids=list(range(N_CORES)), **spmd_kwargs
    )
    y = np.concatenate([res.results[c]["y"] for c in range(N_CORES)], axis=0)
    y = y.astype(np.float32)
    return y.reshape(b, s, weight.shape[0]), res


def kernel(x: np.ndarray, weight: np.ndarray) -> np.ndarray:
    y, _ = _run(x, weight)
    return y
